# revision 42
# baseline (speedup 1.0000x reference)
"""Multi-head causal self-attention (S=4096, D=2048, H=16) on 8 trn2 NeuronCores.

v6: fully fused projection/attention pipeline, softmax denominator off the PE.
(fp8 DoubleRow was evaluated and rejected: e4m3 operand quantization is ~3.6%
rms, giving ~3e-2 rel_l2 per fp8 matmul stage - over the 2e-2 gate.)

Sharding: tensor-parallel over heads - 2 heads per core; host sums the 8
partial out-projections (bias bo and the bv rank-1 term added on host, valid
because softmax rows sum to 1).

Per-core structure (all matmuls bf16, PSUM f32):
  - Projections are 512-row chunks split into six 1-bank sub-blocks
    (q_h0,q_h1,k_h0,k_h1,v_a,v_b) queued as closures and drained one per ~2
    attention pairs: attention alone is ScalarE-bound (one exp per pair >=
    the pair's PE time), so the PE-only projection work is spread at fine
    granularity to keep every region PE-bound.
  - The two heads of an attention chunk are interleaved pair-by-pair, which
    doubles the PE work between a pair's QK and its PV and covers the ~1us
    QK->sem->exp->sem latency with only 4 shared score banks.
  - ScalarE runs ONLY exps; all PSUM drains/copies go to VectorE (anything
    queued on ScalarE delays an exp whose PV the PE is already waiting on).
  - Denominator: exp tiles accumulate elementwise on VectorE into two
    alternating bf16 lanes (breaking the RAW chain), folded, then one
    ones-stationary matmul per (chunk, head) broadcasts the cross-partition
    sum into PSUM: removes the per-pair ones-matmul (~55us of PE).
  - Diagonal pair: QK writes a compact [128,384] region (u1 shifted), one
    smaller exp + mask-mul at chunk start; its PV runs last; its denominator
    contribution folds in at pair 2, off the end-of-chunk critical path.
  - The ones-matmul + reciprocal + normalize for a chunk are deferred into
    the next chunk's prefetch window (before any PV reuses the od banks);
    out-projection pieces trail one chunk and are rationed so the
    projection-free tail chunks still have PE work.
  - PSUM: 4-buffer shared ring ([128,2,256] banks) serves projection
    accumulators AND score tiles (lifetimes alternate); 2 o/den banks, 2
    out-proj banks.
  - Warmup: ~44 dummy ones x ones matmuls ramp the PE clock (starts at half
    rate, doubles after ~4us busy) while the first split DMAs stream in.
"""

import numpy as np
import ml_dtypes

S, D, H = 4096, 2048, 16
HD = D // H  # 128
N_CORES = 8
HPC = H // N_CORES  # heads per core = 2
DPC = HPC * HD  # head dims per core = 256
SCALE = 1.0 / np.sqrt(np.float32(HD))

C = 256  # chunk rows
NC = S // C  # 16 chunks
NDT = D // 128  # 16 d tiles
NKT = S // 128  # 32 sk tiles

_CACHE = {}


def _build():
    import concourse.bacc as bacc
    import concourse.mybir as mybir
    import concourse.tile as tile

    f32 = mybir.dt.float32
    bf = mybir.dt.bfloat16
    Exp = mybir.ActivationFunctionType.Exp
    Copy = mybir.ActivationFunctionType.Copy
    Identity = mybir.ActivationFunctionType.Identity

    nc = bacc.Bacc("TRN2", target_bir_lowering=False)

    xT = nc.dram_tensor("xT", [D, S], bf, kind="ExternalInput")
    wq = nc.dram_tensor("wq", [D, DPC], bf, kind="ExternalInput")
    wk = nc.dram_tensor("wk", [D, DPC], bf, kind="ExternalInput")
    wv = nc.dram_tensor("wv", [D, DPC], bf, kind="ExternalInput")
    wo = nc.dram_tensor("wo", [DPC, D], bf, kind="ExternalInput")
    bqk = nc.dram_tensor("bqk", [2, DPC], f32, kind="ExternalInput")
    masks = nc.dram_tensor("masks", [128, 384], bf, kind="ExternalInput")
    out = nc.dram_tensor("out", [S, D], bf, kind="ExternalOutput")

    xT3 = xT.rearrange("(dt p) s -> p dt s", p=128)
    out3 = out.rearrange("(st p) d -> p st d", p=128)

    with tile.TileContext(nc) as tc:
        with (
            tc.tile_pool(name="persist", bufs=1) as persist,
            tc.tile_pool(name="xin", bufs=3) as xin,
            tc.tile_pool(name="expp", bufs=12) as expp,
            tc.tile_pool(name="lanep", bufs=2) as lanep,
            tc.tile_pool(name="accfp", bufs=2) as accfp,
            tc.tile_pool(name="otp", bufs=5) as otp,
            tc.tile_pool(name="obp", bufs=4) as obp,
            tc.tile_pool(name="rdp", bufs=2) as rdp,
            tc.tile_pool(name="pA", bufs=5, space="PSUM") as pA,
            tc.tile_pool(name="pOD", bufs=2, space="PSUM") as pOD,
            tc.tile_pool(name="pOP", bufs=1, space="PSUM") as pOP,
        ):
            qT = persist.tile([128, HPC, S], bf, tag="qT")
            kT = persist.tile([128, HPC, S], bf, tag="kT")
            vn = persist.tile([128, NKT, DPC], bf, tag="vn")
            wq_sb = persist.tile([128, NDT, DPC], bf, tag="wq")
            wk_sb = persist.tile([128, NDT, DPC], bf, tag="wk")
            wv_sb = persist.tile([128, NDT, DPC], bf, tag="wv")
            wo_sb = persist.tile([128, HPC, D], bf, tag="wo")
            mask_sb = persist.tile([128, 384], bf, tag="mask")
            bias_sb = persist.tile([128, 2, HPC], f32, tag="bias")
            ones_bf = persist.tile([128, 128], bf, tag="ones")

            nc.vector.memset(ones_bf[:], 1.0)

            # ---- initial DMAs: minimal set for the first matmuls first ----
            CP = 2 * C  # 512-wide projection chunks (8 of them)
            NP = S // CP
            xts = {}

            def dma_x(p, lo=0, hi=NDT):
                if p not in xts:
                    xts[p] = xin.tile([128, NDT, CP], bf, tag="xt", name=f"xt{p}")
                nc.sync.dma_start(
                    out=xts[p][:, lo:hi, :], in_=xT3[:, lo:hi, p * CP : (p + 1) * CP]
                )

            def dma_w(w_sb, w_dram, lo, hi):
                nc.sync.dma_start(
                    out=w_sb[:, lo:hi, :],
                    in_=w_dram.rearrange("(dt p) m -> p dt m", p=128)[:, lo:hi, :],
                )

            # PE p-state warmup: the clock starts at half rate and doubles
            # after ~4us of sustained activity. Dummy ones x ones matmuls
            # during the initial DMA window get the ramp done before real
            # work arrives.
            warm = pOP.tile([128, 512], f32, tag="op", name="warm")
            for _ in range(44):
                nc.tensor.matmul(warm[:, 0:128], ones_bf[:], ones_bf[:],
                                 start=True, stop=True)

            # initial DMAs ordered/split by first consumption (issues are
            # ~700ns each, serial on Sync; splitting engages more DMA lanes
            # in parallel during the slow early-transfer window)
            dma_x(0, 0, 4)
            dma_w(wq_sb, wq, 0, 8)
            dma_x(0, 4, 8)
            dma_w(wq_sb, wq, 8, NDT)
            dma_x(0, 8, 12)
            dma_x(0, 12, NDT)
            dma_w(wk_sb, wk, 0, 8)
            dma_w(wk_sb, wk, 8, NDT)
            nc.sync.dma_start(
                out=bias_sb[:], in_=bqk.rearrange("b (h p) -> p b h", p=128)
            )
            dma_w(wv_sb, wv, 0, NDT)
            dma_x(1)
            nc.sync.dma_start(out=mask_sb[:], in_=masks[:, :])
            nc.sync.dma_start(
                out=wo_sb[:], in_=wo.rearrange("(h p) d -> p h d", p=128)
            )

            # ---- deferred work queues ----
            pieces = []  # out-projection piece closures
            pend_den = []  # deferred (ones-matmul + recip + normalize)

            def drain_piece():
                if pieces:
                    pieces.pop(0)()

            def drain_den():
                while pend_den:
                    pend_den.pop(0)()

            # Projection chunk p (512 rows = attention chunks 2p, 2p+1) is six
            # 1-bank sub-blocks (q_h0,q_h1,k_h0,k_h1,v_a,v_b). They are queued
            # as closures and drained one per few attention pairs: attention
            # alone is ScalarE-bound (exp 570ns/pair >= PE 500-870ns/pair), so
            # the PE-only projection work must be spread at fine granularity
            # to keep every region PE-bound.
            pblocks = []  # (p, closure)

            def _qk_block(p, b, h, w_sb, dst):
                xt = xts[p]
                sq = slice(p * CP, (p + 1) * CP)
                ps = pA.tile([128, 2, C], f32, tag="b1", name=f"ps{b}{h}{p}")
                for dt in range(NDT):
                    nc.tensor.matmul(
                        ps[:],
                        w_sb[:, dt, h * 128 : h * 128 + 128],
                        xt[:, dt, :],
                        start=(dt == 0),
                        stop=(dt == NDT - 1),
                    )
                # ScalarE is kept exp-only: any activation queued there delays
                # an exp whose PV the PE is waiting on.
                nc.vector.tensor_scalar_add(dst[:, h, sq], ps[:],
                                            bias_sb[:, b, h : h + 1])

            def _v_block(p, half):
                xt = xts[p]
                ps_v = pA.tile([128, 2, C], f32, tag="b1", name=f"psv{half}{p}")
                for dt in range(NDT):
                    for i in range(2):
                        t = 2 * half + i
                        nc.tensor.matmul(
                            ps_v[:, i, :],
                            xt[:, dt, t * 128 : t * 128 + 128],
                            wv_sb[:, dt, :],
                            start=(dt == 0 and i == 0),
                            stop=(dt == NDT - 1),
                            skip_group_check=True,
                        )
                nc.vector.tensor_copy(
                    vn[:, 4 * p + 2 * half : 4 * p + 2 * half + 2, :], ps_v[:]
                )
                if half == 1:
                    xts.pop(p)

            def enqueue_proj(p):
                def first(p=p):
                    if p + 2 < NP:
                        dma_x(p + 2)
                    _qk_block(p, 0, 0, wq_sb, qT)
                pblocks.append((p, first))
                pblocks.append((p, lambda p=p: _qk_block(p, 0, 1, wq_sb, qT)))
                pblocks.append((p, lambda p=p: _qk_block(p, 1, 0, wk_sb, kT)))
                pblocks.append((p, lambda p=p: _qk_block(p, 1, 1, wk_sb, kT)))
                pblocks.append((p, lambda p=p: _v_block(p, 0)))
                pblocks.append((p, lambda p=p: _v_block(p, 1)))

            def drain_pblock():
                if pblocks:
                    pblocks.pop(0)[1]()

            def force_proj(pmax):
                while pblocks and pblocks[0][0] <= pmax:
                    pblocks.pop(0)[1]()

            pair_ctr = [0]

            def outproj(jc, oT):
                # 8 pieces; each drains one [128,512] PSUM bank; DMA per n.
                obs = {}
                for n in range(4):
                    for si in range(2):
                        def piece(jc=jc, oT=oT, n=n, si=si):
                            if si == 0:
                                obs[n] = obp.tile([128, 2, 512], bf, tag="ob",
                                                  name=f"ob{jc}_{n}")
                            ps_p = pOP.tile([128, 512], f32, tag="op")
                            for h in range(HPC):
                                nc.tensor.matmul(
                                    ps_p[:],
                                    oT[:, h, si * 128 : si * 128 + 128],
                                    wo_sb[:, h, n * 512 : n * 512 + 512],
                                    start=(h == 0),
                                    stop=(h == HPC - 1),
                                )
                            ob = obs[n]
                            nc.vector.tensor_copy(ob[:, si, :], ps_p[:])
                            if si == 1:
                                nc.sync.dma_start(
                                    out=out3[:, 2 * jc : 2 * jc + 2,
                                             n * 512 : n * 512 + 512],
                                    in_=ob[:],
                                )
                        pieces.append(piece)

            def attn(jc):
                npairs = jc + 1
                diag = npairs - 1
                others = list(range(npairs - 1))
                sq = slice(jc * C, (jc + 1) * C)
                oT = otp.tile([128, HPC, C], bf, tag="oT")
                # Heads are interleaved pair-by-pair: the PE work between a
                # pair's QK and its PV doubles (both heads' streams), covering
                # the ~1us QK->exp->PV latency with only ring depth 4.
                ods = [pOD.tile([128, 2, C], f32, tag="od", name=f"od{jc}{h}")
                       for h in range(HPC)]

                def qk(h, pi, pool_tile):
                    for u in range(2):
                        nc.tensor.matmul(
                            pool_tile[:, u, :],
                            kT[:, h, (2 * pi + u) * 128 : (2 * pi + u) * 128 + 128],
                            qT[:, h, sq],
                            start=(u == 0),
                            stop=(u == 1),
                            skip_group_check=True,
                        )

                def qk_exp(h, pi):
                    s = pA.tile([128, 2, C], f32, tag="b1", name=f"s{jc}_{h}_{pi}")
                    qk(h, pi, s)
                    e = expp.tile([128, 2, C], bf, tag="ex")
                    nc.scalar.activation(e[:], s[:], Exp, scale=float(SCALE))
                    return e

                def pv(h, pi, e, first, last):
                    if pi == diag:
                        # e is the flat [128,384] masked diag tile: cols
                        # 0:256 = u0, 256:384 = u1 shifted (sq cols 128..)
                        nc.tensor.matmul(
                            ods[h][:, 0, :],
                            vn[:, 2 * pi, h * 128 : h * 128 + 128],
                            e[:, 0:256],
                            start=first,
                            stop=False,
                            skip_group_check=True,
                        )
                        nc.tensor.matmul(
                            ods[h][:, 0, 128:256],
                            vn[:, 2 * pi + 1, h * 128 : h * 128 + 128],
                            e[:, 256:384],
                            start=False,
                            stop=last,
                            skip_group_check=True,
                        )
                        return
                    for u in range(2):
                        nc.tensor.matmul(
                            ods[h][:, 0, :],
                            vn[:, 2 * pi + u, h * 128 : h * 128 + 128],
                            e[:, u, :],
                            start=(first and u == 0),
                            stop=(last and u == 1),
                            skip_group_check=True,
                        )

                # --- per-head denominator lanes (VectorE) ---
                lanes_h = [[None, None] for _ in range(HPC)]
                unpaired_h = [[None] for _ in range(HPC)]
                nadd_h = [[0] for _ in range(HPC)]

                def den_add(h, e):
                    lanes, unpaired, nadd = lanes_h[h], unpaired_h[h], nadd_h[h]
                    if unpaired[0] is None and None in lanes:
                        unpaired[0] = e
                        return
                    if unpaired[0] is not None:
                        li = lanes.index(None)
                        lanes[li] = lanep.tile([128, 2, C], bf,
                                               tag=f"lane{li}",
                                               name=f"lane{li}_{jc}_{h}")
                        nc.vector.tensor_add(lanes[li][:], unpaired[0][:], e[:])
                        unpaired[0] = None
                    else:
                        li = nadd[0] % 2 if lanes[1] is not None else 0
                        nadd[0] += 1
                        nc.vector.tensor_add(lanes[li][:], lanes[li][:], e[:])

                def den_add_diag(h, exf):
                    # diag contribution from the flat [128,384] masked tile:
                    # cols 0:256 -> u0 lane part, 256:384 -> u1 cols 128:256.
                    lanes, unpaired = lanes_h[h], unpaired_h[h]
                    if lanes[0] is None and unpaired[0] is not None:
                        lanes[0] = lanep.tile([128, 2, C], bf, tag="lane0",
                                              name=f"lane0_{jc}_{h}")
                        e0 = unpaired[0]
                        unpaired[0] = None
                        nc.vector.tensor_add(lanes[0][:, 0, :], e0[:, 0, :],
                                             exf[:, 0:256])
                        nc.vector.tensor_copy(lanes[0][:, 1, 0:128],
                                              e0[:, 1, 0:128])
                        nc.vector.tensor_add(lanes[0][:, 1, 128:256],
                                             e0[:, 1, 128:256],
                                             exf[:, 256:384])
                        return
                    if lanes[0] is None:
                        return  # npairs == 1: handled at fold time
                    li = 0
                    nc.vector.tensor_add(lanes[li][:, 0, :], lanes[li][:, 0, :],
                                         exf[:, 0:256])
                    nc.vector.tensor_add(lanes[li][:, 1, 128:256],
                                         lanes[li][:, 1, 128:256],
                                         exf[:, 256:384])

                # prefetch QKs first: PE cover for the deferred ones-matmuls
                exqs = [{} for _ in range(HPC)]
                for pi in others[:2]:
                    for h in range(HPC):
                        exqs[h][pi] = qk_exp(h, pi)

                # the previous chunk's ones-matmul/recip/normalize must be
                # emitted before any PV writes the (ring-reused) od banks; it
                # also goes ahead of this chunk's mask-muls in the DVE queue
                # so the out-proj pieces see oT as early as possible.
                drain_den()

                # diagonal pairs: QK + exp + mask immediately (score banks
                # free right away); their PVs run last with a chunk of slack.
                # The u1 tile is written shifted into [.,1,0:128] so the
                # meaningful scores are one contiguous [128,384] region - one
                # smaller exp, less ScalarE work at chunk start.
                exds = []
                for h in range(HPC):
                    sdg = pA.tile([128, 2, C], f32, tag="b1",
                                  name=f"sdg{jc}_{h}")
                    nc.tensor.matmul(
                        sdg[:, 0, :],
                        kT[:, h, 2 * diag * 128 : 2 * diag * 128 + 128],
                        qT[:, h, sq],
                        start=True, stop=False, skip_group_check=True,
                    )
                    nc.tensor.matmul(
                        sdg[:, 1, 0:128],
                        kT[:, h, (2 * diag + 1) * 128 : (2 * diag + 1) * 128 + 128],
                        qT[:, h, jc * C + 128 : (jc + 1) * C],
                        start=False, stop=True, skip_group_check=True,
                    )
                    sflat = sdg[:].rearrange("p a b -> p (a b)")
                    ed = expp.tile([128, 2, C], bf, tag="ex")
                    edf = ed[:].rearrange("p a b -> p (a b)")
                    nc.scalar.activation(edf[:, 0:384], sflat[:, 0:384], Exp,
                                         scale=float(SCALE))
                    exd = expp.tile([128, 2, C], bf, tag="ex")
                    exf = exd[:].rearrange("p a b -> p (a b)")
                    nc.vector.tensor_mul(exf[:, 0:384], edf[:, 0:384],
                                         mask_sb[:])
                    exds.append(exf)

                proc = others + [diag]
                # the diag den contribution folds in at pair 2 (its data is
                # ready from chunk start) - off the end-of-chunk fold chain
                diag_den_at = 2 if npairs >= 4 else npairs - 1
                for i, pi in enumerate(proc):
                    if i + 2 < len(others):
                        for h in range(HPC):
                            exqs[h][others[i + 2]] = qk_exp(h, others[i + 2])
                    for h in range(HPC):
                        e = exds[h] if pi == diag else exqs[h].pop(pi)
                        pv(h, pi, e, first=(i == 0), last=(i == npairs - 1))
                        if pi != diag:
                            den_add(h, e)
                        if i == diag_den_at:
                            den_add_diag(h, exds[h])
                    pair_ctr[0] += 1
                    # while projection blocks remain the PE is work-rich:
                    # hold out-proj pieces back (half rate) and spend them in
                    # the projection-free tail chunks instead
                    if not pblocks or pair_ctr[0] % 2 == 1:
                        drain_piece()
                        if i >= 1 and len(pieces) > 16:
                            drain_piece()
                    if pair_ctr[0] % 2 == 0:
                        drain_pblock()

                for h in range(HPC):
                    lanes, unpaired = lanes_h[h], unpaired_h[h]
                    # fold lanes -> accf [128, C] bf16
                    accf = accfp.tile([128, C], bf, tag="accf",
                                      name=f"accf{jc}{h}")
                    if lanes[0] is None:  # npairs == 1: diag only
                        exf = exds[h]
                        nc.vector.tensor_copy(accf[:], exf[:, 0:256])
                        nc.vector.tensor_add(accf[:, 128:256],
                                             accf[:, 128:256],
                                             exf[:, 256:384])
                    else:
                        fold_src = lanes[0]
                        if unpaired[0] is not None:
                            nc.vector.tensor_add(fold_src[:], fold_src[:],
                                                 unpaired[0][:])
                        if lanes[1] is not None:
                            nc.vector.tensor_add(fold_src[:], fold_src[:],
                                                 lanes[1][:])
                        nc.vector.tensor_add(accf[:], fold_src[:, 0, :],
                                             fold_src[:, 1, :])

                    def finish(od=ods[h], accf=accf, h=h, oT=oT):
                        nc.tensor.matmul(od[:, 1, :], ones_bf[:], accf[:],
                                         start=False, stop=True,
                                         skip_group_check=True)
                        rd = rdp.tile([128, C], f32, tag="rd")
                        nc.vector.reciprocal_approx_fast(rd[:], od[:, 1, :])
                        nc.vector.tensor_mul(oT[:, h, :], od[:, 0, :], rd[:])

                    pend_den.append(finish)
                outproj(jc, oT)

            # ---- main loop: proj(0)+proj(1) run up front (nothing to hide
            # behind yet); later proj chunks drain one sub-block per ~3
            # attention pairs, with a forced drain before any attn that
            # depends on them. proj(p) is enqueued 4 attention chunks before
            # its first consumer attn(2p). ----
            enqueue_proj(0)
            enqueue_proj(1)
            force_proj(1)
            for jc in range(NC):
                force_proj(jc // 2)
                attn(jc)
                pnew = jc // 2 + 2
                if jc % 2 == 0 and pnew < NP:
                    enqueue_proj(pnew)
            drain_den()
            while pieces:
                drain_piece()
    nc.finalize()
    return nc


def _get_nc():
    if "nc" not in _CACHE:
        _CACHE["nc"] = _build()
    return _CACHE["nc"]


def _host_masks() -> np.ndarray:
    # compact diag mask [128, 384]: cols 0:256 = u0 tile (keep iff c >= p);
    # cols 256:384 = u1 tile shifted by 128 (keep iff c' >= p, c' = c - 128)
    p = np.arange(128)[:, None]
    c0 = np.arange(C)[None, :]
    c1 = np.arange(128)[None, :]
    blocks = [(c0 >= p).astype(np.float32), (c1 >= p).astype(np.float32)]
    return np.ascontiguousarray(np.concatenate(blocks, axis=1))  # [128, 384]


def make_in_maps(inputs: dict) -> list:
    bf = ml_dtypes.bfloat16
    Wq, bq = np.asarray(inputs["Wq"], np.float32), np.asarray(inputs["bq"], np.float32)
    Wk, bk = np.asarray(inputs["Wk"], np.float32), np.asarray(inputs["bk"], np.float32)
    Wv = np.asarray(inputs["Wv"], np.float32)
    Wo = np.asarray(inputs["Wo"], np.float32)
    xT = np.ascontiguousarray(
        np.asarray(inputs["hidden_states"], np.float32).T.astype(bf)
    )
    masks = _host_masks().astype(bf)
    in_maps = []
    for c in range(N_CORES):
        r = slice(c * DPC, (c + 1) * DPC)
        in_maps.append(
            {
                "xT": xT,
                "wq": np.ascontiguousarray(Wq[r, :].T.astype(bf)),
                "wk": np.ascontiguousarray(Wk[r, :].T.astype(bf)),
                "wv": np.ascontiguousarray(Wv[r, :].T.astype(bf)),
                "wo": np.ascontiguousarray(Wo[:, r].T.astype(bf)),
                "bqk": np.stack([bq[r], bk[r]]),
                "masks": masks,
            }
        )
    return in_maps


def kernel(hidden_states, Wq, bq, Wk, bk, Wv, bv, Wo, bo):
    from concourse.bass_utils import run_bass_kernel_spmd

    Wv, bv = np.asarray(Wv, np.float32), np.asarray(bv, np.float32)
    Wo, bo = np.asarray(Wo, np.float32), np.asarray(bo, np.float32)
    in_maps = make_in_maps(
        dict(hidden_states=hidden_states, Wq=Wq, bq=bq, Wk=Wk, bk=bk, Wv=Wv, Wo=Wo)
    )

    nc = _get_nc()
    results = run_bass_kernel_spmd(nc, in_maps, core_ids=list(range(N_CORES))).results

    acc = results[0]["out"].astype(np.float32)
    for c in range(1, N_CORES):
        acc += results[c]["out"].astype(np.float32)
    acc += (bo + bv @ Wo.T)[None, :]
    return acc


# revision 45
# speedup vs baseline: 1.0302x; 1.0302x over previous
"""Multi-head causal self-attention (S=4096, D=2048, H=16) on 8 trn2 NeuronCores.

v6: fully fused projection/attention pipeline, softmax denominator off the PE.
(fp8 DoubleRow was evaluated and rejected: e4m3 operand quantization is ~3.6%
rms, giving ~3e-2 rel_l2 per fp8 matmul stage - over the 2e-2 gate.)

Sharding: tensor-parallel over heads - 2 heads per core; host sums the 8
partial out-projections (bias bo and the bv rank-1 term added on host, valid
because softmax rows sum to 1).

Per-core structure (all matmuls bf16, PSUM f32):
  - Projections are 512-row chunks split into six 1-bank sub-blocks
    (q_h0,q_h1,k_h0,k_h1,v_a,v_b) queued as closures and drained one per ~2
    attention pairs: attention alone is ScalarE-bound (one exp per pair >=
    the pair's PE time), so the PE-only projection work is spread at fine
    granularity to keep every region PE-bound.
  - The two heads of an attention chunk are interleaved pair-by-pair, which
    doubles the PE work between a pair's QK and its PV and covers the ~1us
    QK->sem->exp->sem latency with only 4 shared score banks.
  - ScalarE runs ONLY exps; all PSUM drains/copies go to VectorE (anything
    queued on ScalarE delays an exp whose PV the PE is already waiting on).
  - Denominator: exp tiles accumulate elementwise on VectorE into two
    alternating bf16 lanes (breaking the RAW chain), folded, then one
    ones-stationary matmul per (chunk, head) broadcasts the cross-partition
    sum into PSUM: removes the per-pair ones-matmul (~55us of PE).
  - Diagonal pair: QK writes a compact [128,384] region (u1 shifted), one
    smaller exp + mask-mul at chunk start; its PV runs last; its denominator
    contribution folds in at pair 2, off the end-of-chunk critical path.
  - The ones-matmul + reciprocal + normalize for a chunk are deferred into
    the next chunk's prefetch window (before any PV reuses the od banks);
    out-projection pieces trail one chunk and are rationed so the
    projection-free tail chunks still have PE work.
  - PSUM: 4-buffer shared ring ([128,2,256] banks) serves projection
    accumulators AND score tiles (lifetimes alternate); 2 o/den banks, 2
    out-proj banks.
  - Warmup: ~44 dummy ones x ones matmuls ramp the PE clock (starts at half
    rate, doubles after ~4us busy) while the first split DMAs stream in.
"""

import numpy as np
import ml_dtypes

S, D, H = 4096, 2048, 16
HD = D // H  # 128
N_CORES = 8
HPC = H // N_CORES  # heads per core = 2
DPC = HPC * HD  # head dims per core = 256
SCALE = 1.0 / np.sqrt(np.float32(HD))

C = 256  # chunk rows
NC = S // C  # 16 chunks
NDT = D // 128  # 16 d tiles
NKT = S // 128  # 32 sk tiles

_CACHE = {}


def _build():
    import concourse.bacc as bacc
    import concourse.mybir as mybir
    import concourse.tile as tile

    f32 = mybir.dt.float32
    bf = mybir.dt.bfloat16
    Exp = mybir.ActivationFunctionType.Exp
    Copy = mybir.ActivationFunctionType.Copy
    Identity = mybir.ActivationFunctionType.Identity

    nc = bacc.Bacc("TRN2", target_bir_lowering=False)

    xT = nc.dram_tensor("xT", [D, S], bf, kind="ExternalInput")
    wq = nc.dram_tensor("wq", [D, DPC], bf, kind="ExternalInput")
    wk = nc.dram_tensor("wk", [D, DPC], bf, kind="ExternalInput")
    wv = nc.dram_tensor("wv", [D, DPC], bf, kind="ExternalInput")
    wo = nc.dram_tensor("wo", [DPC, D], bf, kind="ExternalInput")
    bqk = nc.dram_tensor("bqk", [2, DPC], f32, kind="ExternalInput")
    masks = nc.dram_tensor("masks", [128, 384], bf, kind="ExternalInput")
    out = nc.dram_tensor("out", [S, D], bf, kind="ExternalOutput")

    xT3 = xT.rearrange("(dt p) s -> p dt s", p=128)
    out3 = out.rearrange("(st p) d -> p st d", p=128)

    with tile.TileContext(nc) as tc:
        with (
            tc.tile_pool(name="persist", bufs=1) as persist,
            tc.tile_pool(name="xin", bufs=3) as xin,
            tc.tile_pool(name="expp", bufs=12) as expp,
            tc.tile_pool(name="lanep", bufs=2) as lanep,
            tc.tile_pool(name="accfp", bufs=2) as accfp,
            tc.tile_pool(name="otp", bufs=5) as otp,
            tc.tile_pool(name="obp", bufs=4) as obp,
            tc.tile_pool(name="rdp", bufs=2) as rdp,
            tc.tile_pool(name="pA", bufs=4, space="PSUM") as pA,
            tc.tile_pool(name="pOD", bufs=2, space="PSUM") as pOD,
            tc.tile_pool(name="pOP", bufs=2, space="PSUM") as pOP,
        ):
            qT = persist.tile([128, HPC, S], bf, tag="qT")
            kT = persist.tile([128, HPC, S], bf, tag="kT")
            vn = persist.tile([128, NKT, DPC], bf, tag="vn")
            wq_sb = persist.tile([128, NDT, DPC], bf, tag="wq")
            wk_sb = persist.tile([128, NDT, DPC], bf, tag="wk")
            wv_sb = persist.tile([128, NDT, DPC], bf, tag="wv")
            wo_sb = persist.tile([128, HPC, D], bf, tag="wo")
            mask_sb = persist.tile([128, 384], bf, tag="mask")
            bias_sb = persist.tile([128, 2, HPC], f32, tag="bias")
            ones_bf = persist.tile([128, 128], bf, tag="ones")

            nc.vector.memset(ones_bf[:], 1.0)

            # ---- initial DMAs: minimal set for the first matmuls first ----
            CP = 2 * C  # 512-wide projection chunks (8 of them)
            NP = S // CP
            xts = {}

            def dma_x(p, lo=0, hi=NDT):
                if p not in xts:
                    xts[p] = xin.tile([128, NDT, CP], bf, tag="xt", name=f"xt{p}")
                nc.sync.dma_start(
                    out=xts[p][:, lo:hi, :], in_=xT3[:, lo:hi, p * CP : (p + 1) * CP]
                )

            def dma_w(w_sb, w_dram, lo, hi):
                nc.sync.dma_start(
                    out=w_sb[:, lo:hi, :],
                    in_=w_dram.rearrange("(dt p) m -> p dt m", p=128)[:, lo:hi, :],
                )

            # PE p-state warmup: the clock starts at half rate and doubles
            # after ~4us of sustained activity. Dummy ones x ones matmuls
            # during the initial DMA window get the ramp done before real
            # work arrives.
            warm = pOP.tile([128, 512], f32, tag="op", name="warm")
            for _ in range(44):
                nc.tensor.matmul(warm[:, 0:128], ones_bf[:], ones_bf[:],
                                 start=True, stop=True)

            # initial DMAs ordered/split by first consumption (issues are
            # ~700ns each, serial on Sync; splitting engages more DMA lanes
            # in parallel during the slow early-transfer window)
            dma_x(0, 0, 4)
            dma_w(wq_sb, wq, 0, 8)
            dma_x(0, 4, 8)
            dma_w(wq_sb, wq, 8, NDT)
            dma_x(0, 8, 12)
            dma_x(0, 12, NDT)
            dma_w(wk_sb, wk, 0, 8)
            dma_w(wk_sb, wk, 8, NDT)
            nc.sync.dma_start(
                out=bias_sb[:], in_=bqk.rearrange("b (h p) -> p b h", p=128)
            )
            dma_w(wv_sb, wv, 0, NDT)
            dma_x(1)
            nc.sync.dma_start(out=mask_sb[:], in_=masks[:, :])
            nc.sync.dma_start(
                out=wo_sb[:], in_=wo.rearrange("(h p) d -> p h d", p=128)
            )

            # ---- deferred work queues ----
            pieces = []  # out-projection piece closures
            pend_den = []  # deferred (ones-matmul + recip + normalize)

            def drain_piece():
                if pieces:
                    pieces.pop(0)()

            def drain_den():
                while pend_den:
                    pend_den.pop(0)()

            # Projection chunk p (512 rows = attention chunks 2p, 2p+1) is six
            # 1-bank sub-blocks (q_h0,q_h1,k_h0,k_h1,v_a,v_b). They are queued
            # as closures and drained one per few attention pairs: attention
            # alone is ScalarE-bound (exp 570ns/pair >= PE 500-870ns/pair), so
            # the PE-only projection work must be spread at fine granularity
            # to keep every region PE-bound.
            pblocks = []  # (p, closure)

            def _qk_block(p, b, h, w_sb, dst):
                xt = xts[p]
                sq = slice(p * CP, (p + 1) * CP)
                ps = pA.tile([128, 2, C], f32, tag="b1", name=f"ps{b}{h}{p}")
                for dt in range(NDT):
                    nc.tensor.matmul(
                        ps[:],
                        w_sb[:, dt, h * 128 : h * 128 + 128],
                        xt[:, dt, :],
                        start=(dt == 0),
                        stop=(dt == NDT - 1),
                    )
                # ScalarE is kept exp-only: any activation queued there delays
                # an exp whose PV the PE is waiting on.
                nc.vector.tensor_scalar_add(dst[:, h, sq], ps[:],
                                            bias_sb[:, b, h : h + 1])

            def _v_block(p, half):
                xt = xts[p]
                ps_v = pA.tile([128, 2, C], f32, tag="b1", name=f"psv{half}{p}")
                for dt in range(NDT):
                    for i in range(2):
                        t = 2 * half + i
                        nc.tensor.matmul(
                            ps_v[:, i, :],
                            xt[:, dt, t * 128 : t * 128 + 128],
                            wv_sb[:, dt, :],
                            start=(dt == 0 and i == 0),
                            stop=(dt == NDT - 1),
                            skip_group_check=True,
                        )
                nc.vector.tensor_copy(
                    vn[:, 4 * p + 2 * half : 4 * p + 2 * half + 2, :], ps_v[:]
                )
                if half == 1:
                    xts.pop(p)

            def enqueue_proj(p):
                def first(p=p):
                    if p + 2 < NP:
                        dma_x(p + 2)
                    _qk_block(p, 0, 0, wq_sb, qT)
                pblocks.append((p, first))
                pblocks.append((p, lambda p=p: _qk_block(p, 0, 1, wq_sb, qT)))
                pblocks.append((p, lambda p=p: _qk_block(p, 1, 0, wk_sb, kT)))
                pblocks.append((p, lambda p=p: _qk_block(p, 1, 1, wk_sb, kT)))
                pblocks.append((p, lambda p=p: _v_block(p, 0)))
                pblocks.append((p, lambda p=p: _v_block(p, 1)))

            def drain_pblock():
                if pblocks:
                    pblocks.pop(0)[1]()

            def force_proj(pmax):
                while pblocks and pblocks[0][0] <= pmax:
                    pblocks.pop(0)[1]()

            pair_ctr = [0]

            def outproj(jc, oT):
                # 8 pieces; each drains one [128,512] PSUM bank; DMA per n.
                obs = {}
                for n in range(4):
                    for si in range(2):
                        def piece(jc=jc, oT=oT, n=n, si=si):
                            if si == 0:
                                obs[n] = obp.tile([128, 2, 512], bf, tag="ob",
                                                  name=f"ob{jc}_{n}")
                            ps_p = pOP.tile([128, 512], f32, tag="op")
                            for h in range(HPC):
                                nc.tensor.matmul(
                                    ps_p[:],
                                    oT[:, h, si * 128 : si * 128 + 128],
                                    wo_sb[:, h, n * 512 : n * 512 + 512],
                                    start=(h == 0),
                                    stop=(h == HPC - 1),
                                )
                            ob = obs[n]
                            nc.vector.tensor_copy(ob[:, si, :], ps_p[:])
                            if si == 1:
                                nc.sync.dma_start(
                                    out=out3[:, 2 * jc : 2 * jc + 2,
                                             n * 512 : n * 512 + 512],
                                    in_=ob[:],
                                )
                        pieces.append(piece)

            def attn(jc):
                npairs = jc + 1
                diag = npairs - 1
                others = list(range(npairs - 1))
                sq = slice(jc * C, (jc + 1) * C)
                oT = otp.tile([128, HPC, C], bf, tag="oT")
                # Heads are interleaved pair-by-pair: the PE work between a
                # pair's QK and its PV doubles (both heads' streams), covering
                # the ~1us QK->exp->PV latency with only ring depth 4.
                ods = [pOD.tile([128, 2, C], f32, tag="od", name=f"od{jc}{h}")
                       for h in range(HPC)]

                def qk(h, pi, pool_tile):
                    for u in range(2):
                        nc.tensor.matmul(
                            pool_tile[:, u, :],
                            kT[:, h, (2 * pi + u) * 128 : (2 * pi + u) * 128 + 128],
                            qT[:, h, sq],
                            start=(u == 0),
                            stop=(u == 1),
                            skip_group_check=True,
                        )

                def qk_exp(h, pi):
                    s = pA.tile([128, 2, C], f32, tag="b1", name=f"s{jc}_{h}_{pi}")
                    qk(h, pi, s)
                    e = expp.tile([128, 2, C], bf, tag="ex")
                    nc.scalar.activation(e[:], s[:], Exp, scale=float(SCALE))
                    return e

                def pv(h, pi, e, first, last):
                    if pi == diag:
                        # e is the flat [128,384] masked diag tile: cols
                        # 0:256 = u0, 256:384 = u1 shifted (sq cols 128..)
                        nc.tensor.matmul(
                            ods[h][:, 0, :],
                            vn[:, 2 * pi, h * 128 : h * 128 + 128],
                            e[:, 0:256],
                            start=first,
                            stop=False,
                            skip_group_check=True,
                        )
                        nc.tensor.matmul(
                            ods[h][:, 0, 128:256],
                            vn[:, 2 * pi + 1, h * 128 : h * 128 + 128],
                            e[:, 256:384],
                            start=False,
                            stop=last,
                            skip_group_check=True,
                        )
                        return
                    for u in range(2):
                        nc.tensor.matmul(
                            ods[h][:, 0, :],
                            vn[:, 2 * pi + u, h * 128 : h * 128 + 128],
                            e[:, u, :],
                            start=(first and u == 0),
                            stop=(last and u == 1),
                            skip_group_check=True,
                        )

                # --- per-head denominator lanes (VectorE) ---
                lanes_h = [[None, None] for _ in range(HPC)]
                unpaired_h = [[None] for _ in range(HPC)]
                nadd_h = [[0] for _ in range(HPC)]

                def den_add(h, e):
                    lanes, unpaired, nadd = lanes_h[h], unpaired_h[h], nadd_h[h]
                    if unpaired[0] is None and None in lanes:
                        unpaired[0] = e
                        return
                    if unpaired[0] is not None:
                        li = lanes.index(None)
                        lanes[li] = lanep.tile([128, 2, C], bf,
                                               tag=f"lane{li}",
                                               name=f"lane{li}_{jc}_{h}")
                        nc.vector.tensor_add(lanes[li][:], unpaired[0][:], e[:])
                        unpaired[0] = None
                    else:
                        li = nadd[0] % 2 if lanes[1] is not None else 0
                        nadd[0] += 1
                        nc.vector.tensor_add(lanes[li][:], lanes[li][:], e[:])

                def den_add_diag(h, exf):
                    # diag contribution from the flat [128,384] masked tile:
                    # cols 0:256 -> u0 lane part, 256:384 -> u1 cols 128:256.
                    lanes, unpaired = lanes_h[h], unpaired_h[h]
                    if lanes[0] is None and unpaired[0] is not None:
                        lanes[0] = lanep.tile([128, 2, C], bf, tag="lane0",
                                              name=f"lane0_{jc}_{h}")
                        e0 = unpaired[0]
                        unpaired[0] = None
                        nc.vector.tensor_add(lanes[0][:, 0, :], e0[:, 0, :],
                                             exf[:, 0:256])
                        nc.vector.tensor_copy(lanes[0][:, 1, 0:128],
                                              e0[:, 1, 0:128])
                        nc.vector.tensor_add(lanes[0][:, 1, 128:256],
                                             e0[:, 1, 128:256],
                                             exf[:, 256:384])
                        return
                    if lanes[0] is None:
                        return  # npairs == 1: handled at fold time
                    li = 0
                    nc.vector.tensor_add(lanes[li][:, 0, :], lanes[li][:, 0, :],
                                         exf[:, 0:256])
                    nc.vector.tensor_add(lanes[li][:, 1, 128:256],
                                         lanes[li][:, 1, 128:256],
                                         exf[:, 256:384])

                # prefetch QKs first: PE cover for the deferred ones-matmuls
                exqs = [{} for _ in range(HPC)]
                for pi in others[:2]:
                    for h in range(HPC):
                        exqs[h][pi] = qk_exp(h, pi)

                # the previous chunk's ones-matmul/recip/normalize must be
                # emitted before any PV writes the (ring-reused) od banks; it
                # also goes ahead of this chunk's mask-muls in the DVE queue
                # so the out-proj pieces see oT as early as possible.
                drain_den()

                # diagonal pairs: QK writes a compact [128,384] region (u1
                # shifted into [.,1,0:128]) -> one smaller exp + mask; the
                # diag PV runs last with a chunk of slack. For big chunks the
                # emission is deferred to the end of pair 0, when the p0
                # score banks are provably free (their exps were consumed) -
                # the 4-buffer ring is never over-subscribed.
                exds = []

                def emit_diag():
                    for h in range(HPC):
                        sdg = pA.tile([128, 2, C], f32, tag="b1",
                                      name=f"sdg{jc}_{h}")
                        nc.tensor.matmul(
                            sdg[:, 0, :],
                            kT[:, h, 2 * diag * 128 : 2 * diag * 128 + 128],
                            qT[:, h, sq],
                            start=True, stop=False, skip_group_check=True,
                        )
                        nc.tensor.matmul(
                            sdg[:, 1, 0:128],
                            kT[:, h,
                               (2 * diag + 1) * 128 : (2 * diag + 1) * 128 + 128],
                            qT[:, h, jc * C + 128 : (jc + 1) * C],
                            start=False, stop=True, skip_group_check=True,
                        )
                        sflat = sdg[:].rearrange("p a b -> p (a b)")
                        ed = expp.tile([128, 2, C], bf, tag="ex")
                        edf = ed[:].rearrange("p a b -> p (a b)")
                        nc.scalar.activation(edf[:, 0:384], sflat[:, 0:384],
                                             Exp, scale=float(SCALE))
                        exd = expp.tile([128, 2, C], bf, tag="ex")
                        exf = exd[:].rearrange("p a b -> p (a b)")
                        nc.vector.tensor_mul(exf[:, 0:384], edf[:, 0:384],
                                             mask_sb[:])
                        exds.append(exf)

                if npairs <= 2:
                    emit_diag()

                proc = others + [diag]
                # the diag den contribution folds in at pair 2 (its data is
                # ready from chunk start) - off the end-of-chunk fold chain
                diag_den_at = 2 if npairs >= 4 else npairs - 1
                for i, pi in enumerate(proc):
                    if i + 2 < len(others):
                        for h in range(HPC):
                            exqs[h][others[i + 2]] = qk_exp(h, others[i + 2])
                    for h in range(HPC):
                        e = exds[h] if pi == diag else exqs[h].pop(pi)
                        pv(h, pi, e, first=(i == 0), last=(i == npairs - 1))
                        if pi != diag:
                            den_add(h, e)
                    if i == 0 and npairs > 2:
                        emit_diag()
                    if i == diag_den_at:
                        for h in range(HPC):
                            den_add_diag(h, exds[h])
                    pair_ctr[0] += 1
                    # while projection blocks remain the PE is work-rich:
                    # hold out-proj pieces back (half rate) and spend them in
                    # the projection-free tail chunks instead
                    if not pblocks or pair_ctr[0] % 2 == 1:
                        drain_piece()
                        if i >= 1 and len(pieces) > 16:
                            drain_piece()
                    if pair_ctr[0] % 2 == 0:
                        drain_pblock()

                for h in range(HPC):
                    lanes, unpaired = lanes_h[h], unpaired_h[h]
                    # fold lanes -> accf [128, C] bf16
                    accf = accfp.tile([128, C], bf, tag="accf",
                                      name=f"accf{jc}{h}")
                    if lanes[0] is None:  # npairs == 1: diag only
                        exf = exds[h]
                        nc.vector.tensor_copy(accf[:], exf[:, 0:256])
                        nc.vector.tensor_add(accf[:, 128:256],
                                             accf[:, 128:256],
                                             exf[:, 256:384])
                    else:
                        fold_src = lanes[0]
                        if unpaired[0] is not None:
                            nc.vector.tensor_add(fold_src[:], fold_src[:],
                                                 unpaired[0][:])
                        if lanes[1] is not None:
                            nc.vector.tensor_add(fold_src[:], fold_src[:],
                                                 lanes[1][:])
                        nc.vector.tensor_add(accf[:], fold_src[:, 0, :],
                                             fold_src[:, 1, :])

                    def finish(od=ods[h], accf=accf, h=h, oT=oT):
                        nc.tensor.matmul(od[:, 1, :], ones_bf[:], accf[:],
                                         start=False, stop=True,
                                         skip_group_check=True)
                        rd = rdp.tile([128, C], f32, tag="rd")
                        nc.vector.reciprocal_approx_fast(rd[:], od[:, 1, :])
                        nc.vector.tensor_mul(oT[:, h, :], od[:, 0, :], rd[:])

                    pend_den.append(finish)
                outproj(jc, oT)

            # ---- main loop: proj(0)+proj(1) run up front (nothing to hide
            # behind yet); later proj chunks drain one sub-block per ~3
            # attention pairs, with a forced drain before any attn that
            # depends on them. proj(p) is enqueued 4 attention chunks before
            # its first consumer attn(2p). ----
            enqueue_proj(0)
            enqueue_proj(1)
            force_proj(1)
            for jc in range(NC):
                force_proj(jc // 2)
                attn(jc)
                pnew = jc // 2 + 2
                if jc % 2 == 0 and pnew < NP:
                    enqueue_proj(pnew)
            drain_den()
            while pieces:
                drain_piece()
    nc.finalize()
    return nc


def _get_nc():
    if "nc" not in _CACHE:
        _CACHE["nc"] = _build()
    return _CACHE["nc"]


def _host_masks() -> np.ndarray:
    # compact diag mask [128, 384]: cols 0:256 = u0 tile (keep iff c >= p);
    # cols 256:384 = u1 tile shifted by 128 (keep iff c' >= p, c' = c - 128)
    p = np.arange(128)[:, None]
    c0 = np.arange(C)[None, :]
    c1 = np.arange(128)[None, :]
    blocks = [(c0 >= p).astype(np.float32), (c1 >= p).astype(np.float32)]
    return np.ascontiguousarray(np.concatenate(blocks, axis=1))  # [128, 384]


def make_in_maps(inputs: dict) -> list:
    bf = ml_dtypes.bfloat16
    Wq, bq = np.asarray(inputs["Wq"], np.float32), np.asarray(inputs["bq"], np.float32)
    Wk, bk = np.asarray(inputs["Wk"], np.float32), np.asarray(inputs["bk"], np.float32)
    Wv = np.asarray(inputs["Wv"], np.float32)
    Wo = np.asarray(inputs["Wo"], np.float32)
    xT = np.ascontiguousarray(
        np.asarray(inputs["hidden_states"], np.float32).T.astype(bf)
    )
    masks = _host_masks().astype(bf)
    in_maps = []
    for c in range(N_CORES):
        r = slice(c * DPC, (c + 1) * DPC)
        in_maps.append(
            {
                "xT": xT,
                "wq": np.ascontiguousarray(Wq[r, :].T.astype(bf)),
                "wk": np.ascontiguousarray(Wk[r, :].T.astype(bf)),
                "wv": np.ascontiguousarray(Wv[r, :].T.astype(bf)),
                "wo": np.ascontiguousarray(Wo[:, r].T.astype(bf)),
                "bqk": np.stack([bq[r], bk[r]]),
                "masks": masks,
            }
        )
    return in_maps


def kernel(hidden_states, Wq, bq, Wk, bk, Wv, bv, Wo, bo):
    from concourse.bass_utils import run_bass_kernel_spmd

    Wv, bv = np.asarray(Wv, np.float32), np.asarray(bv, np.float32)
    Wo, bo = np.asarray(Wo, np.float32), np.asarray(bo, np.float32)
    in_maps = make_in_maps(
        dict(hidden_states=hidden_states, Wq=Wq, bq=bq, Wk=Wk, bk=bk, Wv=Wv, Wo=Wo)
    )

    nc = _get_nc()
    results = run_bass_kernel_spmd(nc, in_maps, core_ids=list(range(N_CORES))).results

    acc = results[0]["out"].astype(np.float32)
    for c in range(1, N_CORES):
        acc += results[c]["out"].astype(np.float32)
    acc += (bo + bv @ Wo.T)[None, :]
    return acc
